# revision 27
# baseline (speedup 1.0000x reference)
"""GraphSAGE layer on 8 Trainium2 NeuronCores (Bass/Tile).

Sharding: data-parallel over the 50000 target nodes (6250 rows/core), feature
table + weights replicated.

Neighbor gather uses bulk dma_gather (int16 indices -> the 200k-row table is
split into 7 segments of <=32768 rows).  Gathers are batched per (tile-triple,
segment), split at the 1024-index-per-call ucode limit, with a raised SWDGE
descriptor ring (2048 descriptors).  Refs are host-sorted by (group, segment,
tile) and padded to a cross-core-unified per-(group,seg,tile) layout so one
SPMD program serves all cores; targets are dealt round-robin into tiles by
neighbor-segment profile to balance those counts (output un-permuted on the
host afterwards).  Gathered bf16 rows land in
stream order [pos%128, pos//128]; host-provided one-hot selector blocks route
and sum them per target through PE matmuls accumulating in PSUM ->
aggT[feat, target] directly.  Self rows use classic [P,1] indirect DMA.
Then out.T = W1 @ self.T + (W2/25) @ aggT, ReLU+bias on ACT with fused
BN-stat accumulation, AllReduce of (sum, sumsq), BN apply + row L2-normalize
in row-major layout (whole-buffer DVE ops), per-shard output written back.

The program is data-dependent (unified per-(group,seg,tile) ref counts are
baked in); it is rebuilt/cached per layout hash inside kernel().
"""
import hashlib
from contextlib import ExitStack

import numpy as np
import ml_dtypes

import concourse.bacc as bacc
import concourse.bass as bass
import concourse.tile as tile
from concourse import mybir
from concourse.bass_utils import run_bass_kernel_spmd
from concourse.library_config import mlp
from concourse.masks import make_identity
from concourse._compat import cdiv

BN_EPS = 1e-5
NORM_EPS = 1e-6

N_CORES = 8
D = 128
S = 25                   # neighbors
P = 128
SEG = 32768              # int16-addressable table segment
GROUP = 3                # tiles per gather group
RING = 32768             # dynamic DMA scratch: 2048 descriptors

_prog_cache = {}


def build_program(rows_per_core, table_rows, n_cores, n_total, groups_meta,
                  idx16_cols, oh_cols_total):
    n_tiles = (rows_per_core + P - 1) // P

    nc = bacc.Bacc("TRN2", target_bir_lowering=False, num_devices=n_cores,
                   dynamic_dma_scratch_size=RING)
    f32 = mybir.dt.float32
    bf16 = mybir.dt.bfloat16
    feat = nc.dram_tensor("features", [table_rows, D], f32,
                          kind="ExternalInput")
    featb = nc.dram_tensor("featb", [table_rows, D], bf16,
                           kind="ExternalInput")
    sidx = nc.dram_tensor("sidx", [P, n_tiles], mybir.dt.int32,
                          kind="ExternalInput")
    nidx = nc.dram_tensor("nidx", [P, idx16_cols], mybir.dt.int16,
                          kind="ExternalInput")
    ohs = nc.dram_tensor("ohs", [P, oh_cols_total * P], bf16,
                         kind="ExternalInput")
    w1t = nc.dram_tensor("w1t", [D, D], f32, kind="ExternalInput")
    w2ts = nc.dram_tensor("w2ts", [D, D], f32, kind="ExternalInput")
    bvec = nc.dram_tensor("bvec", [D, 1], f32, kind="ExternalInput")
    gvec = nc.dram_tensor("gvec", [D, 1], f32, kind="ExternalInput")
    betav = nc.dram_tensor("betav", [D, 1], f32, kind="ExternalInput")
    out = nc.dram_tensor("out", [rows_per_core, D], f32, kind="ExternalOutput")

    ar_in = nc.dram_tensor("ar_in", [D, 2], f32)
    ar_out = nc.dram_tensor("ar_out", [D, 2], f32, addr_space="Shared")

    with tile.TileContext(nc) as tc:
        with ExitStack() as ctx:
            singles = ctx.enter_context(tc.tile_pool(name="singles", bufs=1))
            gpool = ctx.enter_context(tc.tile_pool(name="gpool", bufs=2))
            opool = ctx.enter_context(tc.tile_pool(name="opool", bufs=2))
            spool = ctx.enter_context(tc.tile_pool(name="spool", bufs=3))
            wpool = ctx.enter_context(tc.tile_pool(name="wpool", bufs=2))
            psum = ctx.enter_context(tc.tile_pool(name="psum", bufs=1,
                                                  space="PSUM"))
            psum2 = ctx.enter_context(tc.tile_pool(name="psum2", bufs=2,
                                                   space="PSUM"))
            psum3 = ctx.enter_context(tc.tile_pool(name="psum3", bufs=3,
                                                   space="PSUM"))

            nc.gpsimd.load_library(mlp)

            sidx_sb = singles.tile([P, n_tiles], mybir.dt.int32)
            nc.sync.dma_start(out=sidx_sb[:], in_=sidx[:])
            nidx_sb = singles.tile([P, idx16_cols], mybir.dt.int16)
            nc.sync.dma_start(out=nidx_sb[:], in_=nidx[:])
            w1t_sb = singles.tile([D, D], f32)
            nc.sync.dma_start(out=w1t_sb[:], in_=w1t[:])
            w2ts_sb = singles.tile([D, D], f32)
            nc.sync.dma_start(out=w2ts_sb[:], in_=w2ts[:])
            b_sb = singles.tile([D, 1], f32)
            nc.sync.dma_start(out=b_sb[:], in_=bvec[:])
            g_sb = singles.tile([D, 1], f32)
            nc.sync.dma_start(out=g_sb[:], in_=gvec[:])
            beta_sb = singles.tile([D, 1], f32)
            nc.sync.dma_start(out=beta_sb[:], in_=betav[:])
            ident = singles.tile([P, P], f32)
            make_identity(nc, ident[:])
            ones_sb = singles.tile([P, P], f32)
            nc.vector.memset(ones_sb[:], 1.0)

            zT = singles.tile([P, n_tiles, P], f32)       # post-relu row-major
            sums = singles.tile([P, n_tiles], f32)
            sumsq = singles.tile([P, n_tiles], f32)

            # ---------------- Phase A -----------------------------------
            for gm in groups_meta:
                tiles = gm["tiles"]
                nblk_g = gm["nblk"]
                oh_off = gm["oh_off"]
                oh_cols = gm["oh_cols"]
                stg = gpool.tile([P, nblk_g, D], bf16, tag="stg")
                oh_sb = opool.tile([P, oh_cols, P], bf16, tag="oh")
                nc.sync.dma_start(
                    out=oh_sb[:, :, :],
                    in_=ohs[:, oh_off * P:(oh_off + oh_cols) * P])
                aggs = {ti: psum3.tile([P, P], f32, tag="agg",
                                       name=f"agg_{tiles[0]}_{ti}")
                        for ti in range(len(tiles))}
                for (seg, subK, coff, boff_s) in gm["gcalls"]:
                    nb = cdiv(subK, P)
                    s0 = seg * SEG
                    s1 = min(table_rows, s0 + SEG)
                    if subK % P:
                        nc.vector.memset(stg[:, boff_s + nb - 1, :], 0.0)
                    nc.gpsimd.dma_gather(
                        stg[:, boff_s:boff_s + nb, :], featb[s0:s1, :],
                        nidx_sb[:, coff:coff + cdiv(subK, 16)], subK, subK, D)
                # matmuls emitted tile-major so each tile's PSUM accumulation
                # group is contiguous (never two groups interleaved)
                per_tile = {ti: [] for ti in range(len(tiles))}
                for ci, (seg, K, coff, blocks, boff) in enumerate(gm["calls"]):
                    for c, blktiles in enumerate(blocks):
                        for (ti, ohc) in blktiles:
                            per_tile[ti].append((boff + c, ohc))
                for ti in range(len(tiles)):
                    mms = per_tile[ti]
                    for i, (blk, ohc) in enumerate(mms):
                        nc.tensor.matmul(
                            aggs[ti][:], stg[:, blk, :],
                            oh_sb[:, ohc - oh_off, :],
                            start=(i == 0),
                            stop=(i == len(mms) - 1))

                for ti, t in enumerate(tiles):
                    aT = wpool.tile([P, P], f32, tag="aT")
                    nc.scalar.copy(out=aT[:], in_=aggs[ti][:])

                    ga = spool.tile([P, D], f32, tag="ga")
                    nc.gpsimd.indirect_dma_start(
                        out=ga[:, :],
                        out_offset=None,
                        in_=feat[:, :],
                        in_offset=bass.IndirectOffsetOnAxis(
                            ap=sidx_sb[:, t:t + 1], axis=0),
                    )
                    pT = psum.tile([P, P], f32, tag="pT")
                    nc.tensor.transpose(out=pT[:], in_=ga[:, :],
                                        identity=ident[:])
                    sT = wpool.tile([P, P], f32, tag="sT")
                    nc.scalar.copy(out=sT[:], in_=pT[:])

                    mm = psum2.tile([P, P], f32, tag="mm")
                    nc.tensor.matmul(mm[:], w1t_sb[:], sT[:], start=True,
                                     stop=False)
                    nc.tensor.matmul(mm[:], w2ts_sb[:], aT[:], start=False,
                                     stop=True)

                    nv = min(P, rows_per_core - t * P)
                    z = wpool.tile([P, P], f32, tag="z")
                    dump = wpool.tile([P, P], f32, tag="dump")
                    if nv == P:
                        nc.scalar.activation(
                            out=z[:], in_=mm[:],
                            func=mybir.ActivationFunctionType.Relu,
                            bias=b_sb[:], scale=1.0,
                            accum_out=sums[:, t:t + 1])
                        nc.scalar.activation(
                            out=dump[:], in_=z[:],
                            func=mybir.ActivationFunctionType.Square,
                            accum_out=sumsq[:, t:t + 1])
                    else:
                        nc.scalar.activation(
                            out=z[:, 0:nv], in_=mm[:, 0:nv],
                            func=mybir.ActivationFunctionType.Relu,
                            bias=b_sb[:], scale=1.0,
                            accum_out=sums[:, t:t + 1])
                        nc.scalar.activation(
                            out=dump[:, 0:nv], in_=z[:, 0:nv],
                            func=mybir.ActivationFunctionType.Square,
                            accum_out=sumsq[:, t:t + 1])
                        nc.vector.memset(z[:, nv:P], 0.0)

                    pT3 = psum.tile([P, P], f32, tag="pT3")
                    nc.tensor.transpose(out=pT3[:], in_=z[:],
                                        identity=ident[:])
                    nc.scalar.copy(out=zT[:, t, :], in_=pT3[:])

            # ---------------- Phase B: global BN stats -------------------
            gstat = singles.tile([P, 2], f32)
            nc.vector.tensor_reduce(out=gstat[:, 0:1], in_=sums[:],
                                    axis=mybir.AxisListType.X,
                                    op=mybir.AluOpType.add)
            nc.vector.tensor_reduce(out=gstat[:, 1:2], in_=sumsq[:],
                                    axis=mybir.AxisListType.X,
                                    op=mybir.AluOpType.add)
            nc.sync.dma_start(out=ar_in[:], in_=gstat[:])
            nc.gpsimd.collective_compute(
                "AllReduce", mybir.AluOpType.add,
                ins=[ar_in[:]],
                outs=[ar_out[:]],
                replica_groups=[list(range(n_cores))],
            )
            gg = singles.tile([P, 2], f32)
            nc.sync.dma_start(out=gg[:], in_=ar_out[:])

            inv_n = 1.0 / float(n_total)
            mu = singles.tile([P, 1], f32)
            nc.vector.tensor_scalar_mul(mu[:], gg[:, 0:1], inv_n)
            ex2 = singles.tile([P, 1], f32)
            nc.vector.tensor_scalar_mul(ex2[:], gg[:, 1:2], inv_n)
            var = singles.tile([P, 1], f32)
            nc.vector.tensor_mul(var[:], mu[:], mu[:])
            nc.vector.tensor_sub(var[:], ex2[:], var[:])
            nc.vector.tensor_scalar_add(var[:], var[:], BN_EPS)
            std = singles.tile([P, 1], f32)
            nc.scalar.sqrt(out=std[:], in_=var[:])
            rstd = singles.tile([P, 1], f32)
            nc.vector.reciprocal(out=rstd[:], in_=std[:])
            gp = singles.tile([P, 1], f32)
            nc.vector.tensor_mul(gp[:], g_sb[:], rstd[:])
            sh = singles.tile([P, 1], f32)
            nc.vector.tensor_mul(sh[:], mu[:], gp[:])
            nc.vector.tensor_sub(sh[:], beta_sb[:], sh[:])

            diag_gp = singles.tile([P, P], f32)
            nc.vector.tensor_scalar_mul(diag_gp[:], ident[:], gp[:])
            diag_sh = singles.tile([P, P], f32)
            nc.vector.tensor_scalar_mul(diag_sh[:], ident[:], sh[:])
            p_gpb = psum2.tile([P, P], f32, tag="mm")
            nc.tensor.matmul(p_gpb[:], ones_sb[:], diag_gp[:],
                             start=True, stop=True)
            gpb = singles.tile([P, P], f32)
            nc.scalar.copy(out=gpb[:], in_=p_gpb[:])
            p_shb = psum2.tile([P, P], f32, tag="mm")
            nc.tensor.matmul(p_shb[:], ones_sb[:], diag_sh[:],
                             start=True, stop=True)
            shb = singles.tile([P, P], f32)
            nc.scalar.copy(out=shb[:], in_=p_shb[:])

            # ---------------- Phase C: BN apply + L2 normalize -----------
            gpbB = gpb[:][:, None, :].broadcast_to([P, n_tiles, P])
            shbB = shb[:][:, None, :].broadcast_to([P, n_tiles, P])
            nc.vector.tensor_mul(zT[:, :, :], zT[:, :, :], gpbB)
            nc.vector.tensor_add(zT[:, :, :], zT[:, :, :], shbB)
            ysq = singles.tile([P, n_tiles, P], f32)
            nc.vector.tensor_mul(ysq[:, :, :], zT[:, :, :], zT[:, :, :])
            n2 = singles.tile([P, n_tiles], f32)
            nc.vector.tensor_reduce(out=n2[:], in_=ysq[:, :, :],
                                    axis=mybir.AxisListType.X,
                                    op=mybir.AluOpType.add)
            nrm2 = singles.tile([P, n_tiles], f32)
            nc.scalar.sqrt(out=nrm2[:], in_=n2[:])
            nc.vector.tensor_scalar_add(nrm2[:], nrm2[:], NORM_EPS)
            rn2 = singles.tile([P, n_tiles], f32)
            nc.vector.reciprocal(out=rn2[:], in_=nrm2[:])
            rnB = rn2[:][:, :, None].broadcast_to([P, n_tiles, P])
            nc.vector.tensor_mul(zT[:, :, :], zT[:, :, :], rnB)
            for t in range(n_tiles):
                nv = min(P, rows_per_core - t * P)
                nc.sync.dma_start(out=out[t * P:t * P + nv, :],
                                  in_=zT[0:nv, t, :])

    nc.compile()
    return nc


def _pack_idx16(lst):
    """int16 array (len k) -> [128, cdiv(k,16)] wrapped-in-16, replicated."""
    k = len(lst)
    cols = cdiv(k, 16)
    flat = np.full(cols * 16, -1, dtype=np.int16)
    flat[:k] = lst
    arr = flat.reshape(cols, 16).T.copy()
    return np.tile(arr, (8, 1))


def kernel(features, self_idx, neigh_idx, W, b, gamma, beta):
    features = np.ascontiguousarray(np.asarray(features, dtype=np.float32))
    featb = features.astype(ml_dtypes.bfloat16)
    self_idx = np.asarray(self_idx).astype(np.int64)
    neigh_idx = np.asarray(neigh_idx).astype(np.int64)
    W = np.asarray(W, dtype=np.float32)
    n, s = neigh_idx.shape
    table_rows, d = features.shape
    n_cores = N_CORES
    rows_per_core = n // n_cores
    n_tiles = (rows_per_core + P - 1) // P
    pad_rows = n_tiles * P
    n_segs = cdiv(table_rows, SEG)
    n_groups = cdiv(n_tiles, GROUP)

    w1t = np.ascontiguousarray(W[:, :d].T)
    w2ts = np.ascontiguousarray((W[:, d:] / float(s)).T)
    bvec = np.asarray(b, dtype=np.float32).reshape(d, 1).copy()
    gvec = np.asarray(gamma, dtype=np.float32).reshape(d, 1).copy()
    betav = np.asarray(beta, dtype=np.float32).reshape(d, 1).copy()

    allidx = np.concatenate([self_idx[:, None], neigh_idx], axis=1)

    # Balance per-(tile, segment) ref counts: deal targets (sorted by their
    # neighbor-segment profile) round-robin across tiles.  This flattens the
    # per-core count distributions so the cross-core-unified maxk padding
    # shrinks.  perm[slot] = core-local target held at that slot; the output
    # is un-permuted after the device run.
    perms = []
    padded_list = []
    for c in range(n_cores):
        sl = allidx[c * rows_per_core:(c + 1) * rows_per_core]
        segp = sl[:, 1:] // SEG
        counts = np.stack([(segp == ss).sum(1) for ss in range(n_segs)],
                          axis=1)
        key = np.lexsort(counts.T[::-1])
        slots = []
        for p in range(P):
            for t in range(n_tiles):
                i = t * P + p
                if i < rows_per_core:
                    slots.append(i)
        slots = np.array(slots)
        perm = np.empty(rows_per_core, dtype=np.int64)
        perm[slots] = key
        perms.append(perm)
        padded = np.zeros((pad_rows, 1 + S), dtype=np.int64)
        padded[:rows_per_core] = sl[perm]
        # pad targets get no neighbor refs (negative seg -> never selected)
        padded[rows_per_core:, 1:] = -1
        padded_list.append(padded)

    # per-core neighbor refs, by (group, seg, tile): rel row + target-in-tile
    # core_refs[c][(g, seg, t)] = (rel_rows int16[], tloc int[])
    core_refs = [dict() for _ in range(n_cores)]
    for c in range(n_cores):
        padded = padded_list[c]
        neigh = padded[:, 1:]
        for t in range(n_tiles):
            rows = neigh[t * P:(t + 1) * P]
            tloc = np.repeat(np.arange(P), S)
            flat = rows.reshape(-1)
            seg_of = flat // SEG
            g = t // GROUP
            for seg in range(n_segs):
                sel = np.where(seg_of == seg)[0]
                if len(sel):
                    core_refs[c][(g, seg, t)] = (
                        (flat[sel] - seg * SEG).astype(np.int16), tloc[sel])

    # unified per-(g, seg, t) counts
    maxk = np.zeros((n_groups, n_segs, n_tiles), dtype=np.int64)
    for c in range(n_cores):
        for (g, seg, t), (rel, _) in core_refs[c].items():
            maxk[g, seg, t] = max(maxk[g, seg, t], len(rel))

    # build unified layout + program metadata
    groups_meta = []
    coff = 0          # idx16 col offset
    oh_off = 0        # one-hot block col offset
    # record per (g, seg): K, per-tile position offsets, block->tiles map
    call_layout = {}
    SUBMAX = 1024        # dma_gather hard per-call index limit
    for g in range(n_groups):
        tiles = list(range(g * GROUP, min(n_tiles, (g + 1) * GROUP)))
        calls = []
        gcalls = []
        boff = 0
        oh_off_g = oh_off
        for seg in range(n_segs):
            ks = [int(maxk[g, seg, t]) for t in tiles]
            K = int(sum(ks))
            if K == 0:
                continue
            offs = np.cumsum([0] + ks)       # tile position ranges
            nb = cdiv(K, P)
            blocks = []
            for cblk in range(nb):
                lo, hi = cblk * P, min(K, (cblk + 1) * P)
                blktiles = []
                for ti in range(len(tiles)):
                    if offs[ti] < hi and offs[ti + 1] > lo:
                        blktiles.append((ti, oh_off))
                        oh_off += 1
                blocks.append(blktiles)
            calls.append((seg, K, None, blocks, boff))
            subs = []
            for a in range(0, K, SUBMAX):
                bnd = min(K, a + SUBMAX)
                gcalls.append((seg, bnd - a, coff, boff + a // P))
                subs.append((a, bnd, coff))
                coff += cdiv(bnd - a, 16)
            call_layout[(g, seg)] = (K, offs, blocks, subs)
            boff += nb
        groups_meta.append({
            "tiles": tiles,
            "calls": calls,
            "gcalls": gcalls,
            "nblk": boff,
            "oh_off": oh_off_g,
            "oh_cols": oh_off - oh_off_g,
        })
    idx16_cols = coff
    oh_cols_total = oh_off

    # per-core packed inputs
    in_maps = []
    for c in range(n_cores):
        nidx_u = np.full((P, idx16_cols), -1, dtype=np.int16)
        ohs_u = np.zeros((P, oh_cols_total, P), dtype=np.float32)
        for gm in groups_meta:
            tiles = gm["tiles"]
            g = tiles[0] // GROUP
            for (seg, K, co, blocks, boff) in gm["calls"]:
                _, offs, _, subs = call_layout[(g, seg)]
                idxs = np.zeros(K, dtype=np.int16)
                tl = np.full(K, -1, dtype=np.int64)   # -1 = pad (no one-hot)
                for ti, t in enumerate(tiles):
                    rel, tloc = core_refs[c].get((g, seg, t), (None, None))
                    kk = 0 if rel is None else len(rel)
                    o = offs[ti]
                    if kk:
                        idxs[o:o + kk] = rel
                        tl[o:o + kk] = tloc
                for (a, bnd, sco) in subs:
                    nidx_u[:, sco:sco + cdiv(bnd - a, 16)] = _pack_idx16(
                        idxs[a:bnd])
                # one-hot blocks
                for cblk, blktiles in enumerate(blocks):
                    lo, hi = cblk * P, min(K, (cblk + 1) * P)
                    for (ti, ohc) in blktiles:
                        a = max(lo, int(offs[ti]))
                        bnd = min(hi, int(offs[ti + 1]))
                        for j in range(a, bnd):
                            if tl[j] >= 0:
                                ohs_u[j - lo, ohc, tl[j]] = 1.0
        sidx_np = padded_list[c][:, 0].astype(np.int32)
        sidx_arr = np.ascontiguousarray(sidx_np.reshape(n_tiles, P).T.copy())
        in_maps.append({
            "features": features,
            "featb": featb,
            "sidx": sidx_arr,
            "nidx": nidx_u,
            "ohs": np.ascontiguousarray(
                ohs_u.reshape(P, oh_cols_total * P)).astype(
                    ml_dtypes.bfloat16),
            "w1t": w1t,
            "w2ts": w2ts,
            "bvec": bvec,
            "gvec": gvec,
            "betav": betav,
        })

    key = hashlib.sha1(
        maxk.tobytes() + np.int64(rows_per_core).tobytes()
        + np.int64(table_rows).tobytes()).hexdigest()
    if key not in _prog_cache:
        _prog_cache[key] = build_program(
            rows_per_core, table_rows, n_cores, n, groups_meta,
            idx16_cols, oh_cols_total)
    nc = _prog_cache[key]

    global _last_in_maps
    _last_in_maps = in_maps
    res = run_bass_kernel_spmd(nc, in_maps, core_ids=list(range(n_cores)))
    parts = []
    for c in range(n_cores):
        dev = res.results[c]["out"]
        unp = np.empty_like(dev)
        unp[perms[c]] = dev
        parts.append(unp)
    outp = np.concatenate(parts, axis=0)
    return outp


_last_in_maps = None
